# revision 40
# baseline (speedup 1.0000x reference)
"""Multi-head self-attention with SDPA softcap, sharded over 8 NeuronCores.

Sharding: batch x head-group tensor parallel. Core c owns batch c//4 and
heads [4*(c%4), 4*(c%4)+4) (4 of 16 heads, 512 of 2048 dims):
  - pass A: k,v projections for its batch -> SBUF resident (no DRAM spill),
  - pass B: q projection -> SBUF resident (reuses pass-A weight/x rings so
    the wq load overlaps pass-A compute),
  - phase 2: attention (softcap tanh + softmax) per 256-query tile,
    output projection with its row-slice of wo -> partial [S, D] output.
Host sums the 4 partials per batch.

All heavy matmuls run in float32r (fp32 with 11-bit mantissa, full PE rate).
Inputs are pre-rounded to fp32r on the host so device rounding is exact.
"""

import sys

if "/opt/trn_rl_repo" not in sys.path:
    sys.path.insert(0, "/opt/trn_rl_repo")

import numpy as np
import ml_dtypes
_bf16 = ml_dtypes.bfloat16

import concourse.bass as bass
import concourse.bacc as bacc
import concourse.tile as tile
from concourse import mybir
from concourse.bass_utils import run_bass_kernel_spmd

F32 = mybir.dt.float32
F32R = mybir.dt.float32r
BF16 = mybir.dt.bfloat16

D = 2048          # model dim
H = 16            # total heads
DK = 128          # head dim
B = 2
S = 2048
NCORES = 8
HC = 4            # heads per core
DPC = HC * DK     # 512: d' slice per core

KC = D // 128     # 16 contraction chunks over model dim
TCOL = 512        # projection token-column width
NTCOL = S // TCOL             # 4
TQ = 256          # query-tile width
NTQ = S // TQ                 # 8
NTK = S // 128    # 16 key blocks
NHF = NTK // 2    # 8 key blocks per half
KH = KC // 2


def _round_fp32r(x: np.ndarray) -> np.ndarray:
    """Round fp32 to fp32r (11-bit mantissa), round-to-nearest-even."""
    u = np.ascontiguousarray(x, dtype=np.float32).view(np.uint32)
    low = u & np.uint32(0xFFF)
    kept = u & np.uint32(0xFFFFF000)
    half = np.uint32(0x800)
    roundup = (low > half) | ((low == half) & ((kept & np.uint32(0x1000)) != 0))
    out = kept + np.where(roundup, np.uint32(0x1000), np.uint32(0))
    return out.view(np.float32)


def _build_program(cap: float, dbg: bool = False):
    nc = bacc.Bacc("TRN2", target_bir_lowering=False, debug=False,
                   num_devices=NCORES)

    xT = nc.dram_tensor("xT", [D, S], F32R, kind="ExternalInput").ap()
    ones_d = nc.dram_tensor("ones", [128, 128], BF16, kind="ExternalInput").ap()
    wqT = nc.dram_tensor("wqT", [D, DPC], F32R, kind="ExternalInput").ap()
    wkT = nc.dram_tensor("wkT", [D, DPC], F32R, kind="ExternalInput").ap()
    wvT = nc.dram_tensor("wvT", [D, DPC], F32R, kind="ExternalInput").ap()
    woT = nc.dram_tensor("woT", [DPC, D], F32R, kind="ExternalInput").ap()
    biasT = nc.dram_tensor("biasT", [S, S], F32, kind="ExternalInput").ap()
    out_d = nc.dram_tensor("out_partial", [S, D], F32, kind="ExternalOutput").ap()
    if dbg:
        kdump = nc.dram_tensor("kdump", [128, HC, S], BF16, kind="ExternalOutput").ap()
        qdump = nc.dram_tensor("qdump", [128, HC, S], BF16, kind="ExternalOutput").ap()
        vdump = nc.dram_tensor("vdump", [128, NTK, DPC], BF16, kind="ExternalOutput").ap()

    xT_v = xT.rearrange("(kc p) t -> p kc t", p=128)
    biasT_v = biasT.rearrange("(kb p) q -> p kb q", p=128)
    wqT_v = wqT.rearrange("(kc p) n -> p kc n", p=128)
    wkT_v = wkT.rearrange("(kc p) n -> p kc n", p=128)
    wvT_v = wvT.rearrange("(kc p) n -> p kc n", p=128)
    woT_v = woT.rearrange("(h p) n -> p h n", p=128)

    with tile.TileContext(nc) as tc:
        with (
            tc.tile_pool(name="const", bufs=1) as cpool,
            tc.tile_pool(name="resid", bufs=1) as resid,
        ):
            ones_full = cpool.tile([128, 128], BF16)
            nc.sync.dma_start(out=ones_full[:], in_=ones_d[:])

            # SBUF-resident tensors (live through the whole kernel)
            kT_sb = resid.tile([128, HC, S], BF16)        # [dk, h, tok]
            v_sb = resid.tile([128, NTK, DPC], BF16)      # [tok%, tokb, (h dk)]
            q_sb = resid.tile([128, HC, S], BF16)         # [dk, h, tok]

            pbias = tc.tile_pool(name="p3b0", bufs=1)
            p3b0 = pbias.__enter__()
            bias0 = p3b0.tile([128, NHF, TQ], F32, tag="b0")

            # ---------- Passes A+B: k,v then q projections ----------
            with (
                tc.tile_pool(name="p1w", bufs=1) as p1w,
                tc.tile_pool(name="p1x", bufs=4) as p1x,
                tc.tile_pool(name="p1ps", bufs=4, space="PSUM") as p1ps,
                tc.tile_pool(name="p1psv", bufs=1, space="PSUM") as p1psv,
            ):
                wk_a = p1w.tile([128, KH, DPC], F32R, tag="wka")
                wk_b = p1w.tile([128, KH, DPC], F32R, tag="wkb")
                wv_a = p1w.tile([128, KH, DPC], F32R, tag="wva")
                wv_b = p1w.tile([128, KH, DPC], F32R, tag="wvb")
                nc.sync.dma_start(out=wk_a[:], in_=wkT_v[:, 0:KH, :])
                nc.sync.dma_start(out=wk_b[:], in_=wkT_v[:, KH:KC, :])
                nc.scalar.dma_start(out=wv_a[:], in_=wvT_v[:, 0:KH, :])
                nc.scalar.dma_start(out=wv_b[:], in_=wvT_v[:, KH:KC, :])

                def proj_col(tcol, w_a, w_b, dst, do_v):
                    t0 = tcol * TCOL
                    # x in quarter-column tiles: fine-grained ring frees let
                    # the next column's loads overlap this column's compute
                    xq = []
                    for j in range(4):
                        xt = p1x.tile([128, 4, TCOL], F32R, tag="x",
                                      name=f"xq{j}")
                        nc.gpsimd.dma_start(
                            out=xt[:], in_=xT_v[:, j * 4:(j + 1) * 4,
                                                t0:t0 + TCOL])
                        xq.append(xt)
                    for m in range(HC):
                        ps = p1ps.tile([128, TCOL], F32, tag="psk")
                        for kc in range(KC):
                            xc = xq[kc // 4][:, kc % 4, :]
                            wc = w_a if kc < KH else w_b
                            nc.tensor.matmul(
                                ps[:],
                                wc[:, kc % KH, m * 128:(m + 1) * 128],
                                xc,
                                start=(kc == 0),
                                stop=(kc == KC - 1),
                            )
                        nc.vector.tensor_copy(dst[:, m, t0:t0 + TCOL], ps[:])
                    if not do_v:
                        return
                    # v: stationary x chunks -> natural [tok, (h dk)] layout.
                    # kc-outer order frees early x quarters mid-column.
                    vps = [p1psv.tile([128, DPC], F32, tag=f"psv{ts}",
                                      name=f"vp{ts}")
                           for ts in range(TCOL // 128)]
                    for kc in range(KC):
                        wc = wv_a if kc < KH else wv_b
                        for ts in range(TCOL // 128):
                            nc.tensor.matmul(
                                vps[ts][:],
                                xq[kc // 4][:, kc % 4,
                                            ts * 128:(ts + 1) * 128],
                                wc[:, kc % KH, :],
                                start=(kc == 0),
                                stop=(kc == KC - 1),
                            )
                    for ts in range(TCOL // 128):
                        nc.vector.tensor_copy(
                            v_sb[:, tcol * (TCOL // 128) + ts, :], vps[ts][:])

                proj_col(0, wk_a, wk_b, kT_sb, do_v=True)
                # prefetch first bias tile (phase 2) behind the column loads
                nc.gpsimd.dma_start(
                    out=bias0[:], in_=biasT_v[:, 0:NHF, 0:TQ])
                for tcol in range(1, NTCOL):
                    proj_col(tcol, wk_a, wk_b, kT_sb, do_v=True)

                # pass B reuses the wk ring slots (dep-ordered, load overlaps
                # the pass-A tail) and the same x ring
                wq_a = p1w.tile([128, KH, DPC], F32R, tag="wka")
                wq_b = p1w.tile([128, KH, DPC], F32R, tag="wkb")
                nc.sync.dma_start(out=wq_a[:], in_=wqT_v[:, 0:KH, :])
                nc.scalar.dma_start(out=wq_b[:], in_=wqT_v[:, KH:KC, :])
                for tcol in range(NTCOL):
                    proj_col(tcol, wq_a, wq_b, q_sb, do_v=False)

            if dbg:
                nc.sync.dma_start(out=kdump[:], in_=kT_sb[:])
                nc.sync.dma_start(out=qdump[:], in_=q_sb[:])
                nc.sync.dma_start(out=vdump[:], in_=v_sb[:])

            # -------- Phase 2: attention + output projection --------
            with (
                tc.tile_pool(name="p3wo", bufs=1) as p3wo,
                tc.tile_pool(name="p3b1", bufs=1) as p3b1,
                tc.tile_pool(name="p3s", bufs=2) as p3s,
                tc.tile_pool(name="p3er", bufs=2) as p3er,
                tc.tile_pool(name="p3ot", bufs=8) as p3ot,
                tc.tile_pool(name="p3recip", bufs=2) as p3recip,
                tc.tile_pool(name="p3out", bufs=2) as p3out,
                tc.tile_pool(name="psS", bufs=2, space="PSUM") as psS,
                tc.tile_pool(name="psAZ", bufs=2, space="PSUM") as psAZ,
                tc.tile_pool(name="psO", bufs=2, space="PSUM") as psO,
            ):
                wo_sb = p3wo.tile([128, HC, D], F32R)
                for hh in range(HC):
                    eng = nc.sync if hh % 2 == 0 else nc.scalar
                    eng.dma_start(out=wo_sb[:, hh, :], in_=woT_v[:, hh, :])

                for qt in range(NTQ):
                    q0 = qt * TQ
                    # bias for this query tile, both key halves
                    bias_h = [None, None]
                    for half in range(2):
                        if qt == 0 and half == 0:
                            bias_h[0] = bias0  # prefetched during pass A
                            continue
                        pool = p3b0 if half == 0 else p3b1
                        bc = pool.tile([128, NHF, TQ], F32, tag=f"b{half}")
                        nc.gpsimd.dma_start(
                            out=bc[:],
                            in_=biasT_v[:, half * NHF:(half + 1) * NHF,
                                        q0:q0 + TQ],
                        )
                        bias_h[half] = bc

                    ot_tiles = []
                    for h in range(HC):
                        qcol = q_sb[:, h, q0:q0 + TQ]
                        s_buf = p3s.tile([128, NTK, TQ], F32, tag="s")
                        # scores: stationary kT blocks, psum pairs -> s_buf
                        for kg in range(NTK // 2):
                            sps = psS.tile([128, 2, TQ], F32, tag="sps")
                            for kk in range(2):
                                kb = kg * 2 + kk
                                nc.tensor.matmul(
                                    sps[:, kk, :],
                                    kT_sb[:, h, kb * 128:(kb + 1) * 128],
                                    qcol,
                                    start=True,
                                    stop=True,
                                )
                            bh = bias_h[kg // 4]
                            nc.vector.tensor_add(
                                s_buf[:, kg * 2:(kg + 1) * 2, :],
                                sps[:],
                                bh[:, (kg % 4) * 2:(kg % 4) * 2 + 2, :],
                            )
                        av = psAZ.tile([128, TQ], F32, tag="av")
                        zb = psAZ.tile([128, TQ], F32, tag="zb")
                        for half in range(2):
                            hs = slice(half * NHF, (half + 1) * NHF)
                            s_flat = s_buf[:, hs, :].rearrange(
                                "p a b -> p (a b)")
                            nc.scalar.activation(
                                s_flat, s_flat,
                                mybir.ActivationFunctionType.Tanh,
                                scale=1.0 / cap,
                            )
                            er = p3er.tile([128, NHF, TQ], BF16, tag="er")
                            nc.scalar.activation(
                                er[:].rearrange("p a b -> p (a b)"),
                                s_flat,
                                mybir.ActivationFunctionType.Exp,
                                scale=cap,
                            )
                            for kk in range(NHF):
                                kb = half * NHF + kk
                                nc.tensor.matmul(
                                    av[:],
                                    v_sb[:, kb, h * DK:(h + 1) * DK],
                                    er[:, kk, :],
                                    start=(kb == 0),
                                    stop=(kb == NTK - 1),
                                )
                                nc.tensor.matmul(
                                    zb[:],
                                    ones_full[:],
                                    er[:, kk, :],
                                    start=(kb == 0),
                                    stop=(kb == NTK - 1),
                                )
                        recip = p3recip.tile([128, TQ], F32, tag="recip")
                        nc.vector.reciprocal_approx_fast(
                            out=recip[:], in_=zb[:])
                        ot = p3ot.tile([128, TQ], F32R, tag="ot")
                        nc.vector.tensor_mul(ot[:], av[:], recip[:])
                        ot_tiles.append(ot)

                    # output projection for this query tile
                    for qs in range(TQ // 128):
                        for dc in range(4):
                            d0 = dc * 512
                            op = psO.tile([128, 512], F32, tag="op")
                            for h in range(HC):
                                nc.tensor.matmul(
                                    op[:],
                                    ot_tiles[h][:, qs * 128:(qs + 1) * 128],
                                    wo_sb[:, h, d0:d0 + 512],
                                    start=(h == 0),
                                    stop=(h == HC - 1),
                                )
                            outt = p3out.tile([128, 512], F32, tag="outt")
                            nc.vector.tensor_copy(outt[:], op[:])
                            nc.gpsimd.dma_start(
                                out=out_d[q0 + qs * 128:q0 + (qs + 1) * 128,
                                          d0:d0 + 512],
                                in_=outt[:],
                            )

            pbias.__exit__(None, None, None)

    nc.compile()
    return nc


_PROGRAM_CACHE: dict = {}


def _get_program(cap: float):
    if cap not in _PROGRAM_CACHE:
        _PROGRAM_CACHE[cap] = _build_program(cap)
    return _PROGRAM_CACHE[cap]


def _prepare_in_maps(inp, wq, wk, wv, wo, attn_bias, softcap):
    inp = np.asarray(inp, dtype=np.float32)
    xTs = [
        _round_fp32r(np.ascontiguousarray(inp[b].T)) for b in range(B)
    ]
    biasT = np.ascontiguousarray(
        np.asarray(attn_bias, dtype=np.float32).reshape(S, S).T
    )
    wq = np.asarray(wq, dtype=np.float32)
    wk = np.asarray(wk, dtype=np.float32)
    wv = np.asarray(wv, dtype=np.float32)
    wo = np.asarray(wo, dtype=np.float32)
    scale = 1.0 / np.sqrt(np.float32(DK))

    in_maps = []
    for c in range(NCORES):
        b = c // 4
        g = c % 4
        rows = slice(g * DPC, (g + 1) * DPC)
        in_maps.append({
            "xT": xTs[b],
            "ones": np.ones((128, 128), dtype=_bf16),
            "wqT": _round_fp32r((wq[rows] * scale).T),
            "wkT": _round_fp32r(wk[rows].T),
            "wvT": _round_fp32r(wv[rows].T),
            "woT": _round_fp32r(wo[:, rows].T),
            "biasT": biasT,
        })
    return in_maps


def run(inputs: dict, trace: bool = False):
    """Run the SPMD kernel. Returns (full_output, BassKernelResults)."""
    cap = float(inputs["softcap"])
    nc = _get_program(cap)
    in_maps = _prepare_in_maps(
        inputs["inp"], inputs["wq"], inputs["wk"], inputs["wv"],
        inputs["wo"], inputs["attn_bias"], inputs["softcap"],
    )
    res = run_bass_kernel_spmd(
        nc, in_maps, list(range(NCORES)), trace=trace,
    )
    out = np.zeros((B, S, D), dtype=np.float64)
    for c in range(NCORES):
        out[c // 4] += res.results[c]["out_partial"]
    return out.astype(np.float32), res


def kernel(**inputs) -> np.ndarray:
    out, _ = run(inputs, trace=False)
    return out


if __name__ == "__main__":
    rng = np.random.default_rng(0)
    sc = 1.0 / np.sqrt(D)
    inputs = {
        "inp": rng.standard_normal((B, S, D)).astype(np.float32),
        "wq": (rng.standard_normal((D, D)) * sc).astype(np.float32),
        "wk": (rng.standard_normal((D, D)) * sc).astype(np.float32),
        "wv": (rng.standard_normal((D, D)) * sc).astype(np.float32),
        "wo": (rng.standard_normal((D, D)) * sc).astype(np.float32),
        "attn_bias": rng.standard_normal((1, 1, S, S)).astype(np.float32),
        "softcap": 30,
    }
    out = kernel(**inputs)
    print("out", out.shape, out.dtype, float(np.abs(out).max()))


# revision 44
# speedup vs baseline: 1.0161x; 1.0161x over previous
"""Multi-head self-attention with SDPA softcap, sharded over 8 NeuronCores.

Sharding: batch x head-group tensor parallel. Core c owns batch c//4 and
heads [4*(c%4), 4*(c%4)+4) (4 of 16 heads, 512 of 2048 dims):
  - pass A: k,v projections for its batch -> SBUF resident (no DRAM spill),
  - pass B: q projection -> SBUF resident (reuses pass-A weight/x rings so
    the wq load overlaps pass-A compute),
  - phase 2: attention (softcap tanh + softmax) per 256-query tile,
    output projection with its row-slice of wo -> partial [S, D] output.
Host sums the 4 partials per batch.

All heavy matmuls run in float32r (fp32 with 11-bit mantissa, full PE rate).
Inputs are pre-rounded to fp32r on the host so device rounding is exact.
"""

import sys

if "/opt/trn_rl_repo" not in sys.path:
    sys.path.insert(0, "/opt/trn_rl_repo")

import numpy as np
import ml_dtypes
_bf16 = ml_dtypes.bfloat16

import concourse.bass as bass
import concourse.bacc as bacc
import concourse.tile as tile
from concourse import mybir
from concourse.bass_utils import run_bass_kernel_spmd

F32 = mybir.dt.float32
F32R = mybir.dt.float32r
BF16 = mybir.dt.bfloat16

D = 2048          # model dim
H = 16            # total heads
DK = 128          # head dim
B = 2
S = 2048
NCORES = 8
HC = 4            # heads per core
DPC = HC * DK     # 512: d' slice per core

KC = D // 128     # 16 contraction chunks over model dim
TCOL = 1024       # projection token-column width
NTCOL = S // TCOL             # 4
TQ = 512          # query-tile width
NTQ = S // TQ                 # 8
NTK = S // 128    # 16 key blocks
NHF = NTK // 2    # 8 key blocks per half
KH = KC // 2


def _round_fp32r(x: np.ndarray) -> np.ndarray:
    """Round fp32 to fp32r (11-bit mantissa), round-to-nearest-even."""
    u = np.ascontiguousarray(x, dtype=np.float32).view(np.uint32)
    low = u & np.uint32(0xFFF)
    kept = u & np.uint32(0xFFFFF000)
    half = np.uint32(0x800)
    roundup = (low > half) | ((low == half) & ((kept & np.uint32(0x1000)) != 0))
    out = kept + np.where(roundup, np.uint32(0x1000), np.uint32(0))
    return out.view(np.float32)


def _build_program(cap: float, dbg: bool = False):
    nc = bacc.Bacc("TRN2", target_bir_lowering=False, debug=False,
                   num_devices=NCORES)

    xT = nc.dram_tensor("xT", [D, S], F32R, kind="ExternalInput").ap()
    ones_d = nc.dram_tensor("ones", [128, 128], BF16, kind="ExternalInput").ap()
    wqT = nc.dram_tensor("wqT", [D, DPC], F32R, kind="ExternalInput").ap()
    wkT = nc.dram_tensor("wkT", [D, DPC], F32R, kind="ExternalInput").ap()
    wvT = nc.dram_tensor("wvT", [D, DPC], F32R, kind="ExternalInput").ap()
    woT = nc.dram_tensor("woT", [DPC, D], BF16, kind="ExternalInput").ap()
    biasT = nc.dram_tensor("biasT", [S, S], F32, kind="ExternalInput").ap()
    out_d = nc.dram_tensor("out_partial", [S, D], F32, kind="ExternalOutput").ap()
    if dbg:
        kdump = nc.dram_tensor("kdump", [128, HC, S], BF16, kind="ExternalOutput").ap()
        qdump = nc.dram_tensor("qdump", [128, HC, S], BF16, kind="ExternalOutput").ap()
        vdump = nc.dram_tensor("vdump", [128, NTK, DPC], BF16, kind="ExternalOutput").ap()

    xT_v = xT.rearrange("(kc p) t -> p kc t", p=128)
    biasT_v = biasT.rearrange("(kb p) q -> p kb q", p=128)
    wqT_v = wqT.rearrange("(kc p) n -> p kc n", p=128)
    wkT_v = wkT.rearrange("(kc p) n -> p kc n", p=128)
    wvT_v = wvT.rearrange("(kc p) n -> p kc n", p=128)
    woT_v = woT.rearrange("(h p) n -> p h n", p=128)

    with tile.TileContext(nc) as tc:
        with (
            tc.tile_pool(name="const", bufs=1) as cpool,
            tc.tile_pool(name="resid", bufs=1) as resid,
        ):
            ones_full = cpool.tile([128, 128], BF16)
            nc.sync.dma_start(out=ones_full[:], in_=ones_d[:])

            # SBUF-resident tensors (live through the whole kernel)
            kT_sb = resid.tile([128, HC, S], BF16)        # [dk, h, tok]
            v_sb = resid.tile([128, NTK, DPC], BF16)      # [tok%, tokb, (h dk)]
            q_sb = resid.tile([128, HC, S], BF16)         # [dk, h, tok]

            pbias = tc.tile_pool(name="p3b0", bufs=1)
            p3b0 = pbias.__enter__()
            bias0 = p3b0.tile([128, NHF, TQ], F32, tag="b0")

            # ---------- Passes A+B: k,v then q projections ----------
            with (
                tc.tile_pool(name="p1w", bufs=1) as p1w,
                tc.tile_pool(name="p1x", bufs=4) as p1x,
                tc.tile_pool(name="p1ps", bufs=2, space="PSUM") as p1ps,
                tc.tile_pool(name="p1psv", bufs=1, space="PSUM") as p1psv,
            ):
                wk_a = p1w.tile([128, KH, DPC], F32R, tag="wka")
                wk_b = p1w.tile([128, KH, DPC], F32R, tag="wkb")
                wv_a = p1w.tile([128, KH, DPC], F32R, tag="wva")
                wv_b = p1w.tile([128, KH, DPC], F32R, tag="wvb")
                nc.sync.dma_start(out=wk_a[:], in_=wkT_v[:, 0:KH, :])
                nc.sync.dma_start(out=wk_b[:], in_=wkT_v[:, KH:KC, :])
                nc.scalar.dma_start(out=wv_a[:], in_=wvT_v[:, 0:KH, :])
                nc.scalar.dma_start(out=wv_b[:], in_=wvT_v[:, KH:KC, :])

                def proj_col(tcol, w_a, w_b, dst, do_v):
                    t0 = tcol * TCOL
                    # x in quarter-column tiles: fine-grained ring frees let
                    # the next column's loads overlap this column's compute
                    xq = []
                    for j in range(4):
                        xt = p1x.tile([128, 4, TCOL], F32R, tag="x",
                                      name=f"xq{j}")
                        nc.gpsimd.dma_start(
                            out=xt[:], in_=xT_v[:, j * 4:(j + 1) * 4,
                                                t0:t0 + TCOL])
                        xq.append(xt)
                    for m in range(HC):
                        ps = p1ps.tile([128, TCOL], F32, tag="psk")
                        for kc in range(KC):
                            wc = w_a if kc < KH else w_b
                            for ph in range(TCOL // 512):
                                nc.tensor.matmul(
                                    ps[:, ph * 512:(ph + 1) * 512],
                                    wc[:, kc % KH, m * 128:(m + 1) * 128],
                                    xq[kc // 4][:, kc % 4,
                                                ph * 512:(ph + 1) * 512],
                                    start=(kc == 0),
                                    stop=(kc == KC - 1),
                                )
                        nc.vector.tensor_copy(dst[:, m, t0:t0 + TCOL], ps[:])
                    if not do_v:
                        return
                    # v: stationary x chunks -> natural [tok, (h dk)] layout.
                    # kc-outer order (within ts-halves of 4 psum banks)
                    # frees early x quarters mid-column.
                    for tsh in range(TCOL // 512):
                        vps = [p1psv.tile([128, DPC], F32, tag=f"psv{ts}",
                                          name=f"vp{ts}")
                               for ts in range(4)]
                        for kc in range(KC):
                            wc = wv_a if kc < KH else wv_b
                            for ts4 in range(4):
                                ts = tsh * 4 + ts4
                                nc.tensor.matmul(
                                    vps[ts4][:],
                                    xq[kc // 4][:, kc % 4,
                                                ts * 128:(ts + 1) * 128],
                                    wc[:, kc % KH, :],
                                    start=(kc == 0),
                                    stop=(kc == KC - 1),
                                )
                        for ts4 in range(4):
                            nc.vector.tensor_copy(
                                v_sb[:, tcol * (TCOL // 128) + tsh * 4 + ts4,
                                     :],
                                vps[ts4][:])

                proj_col(0, wk_a, wk_b, kT_sb, do_v=True)
                # prefetch first bias tile (phase 2) behind the column loads
                nc.gpsimd.dma_start(
                    out=bias0[:], in_=biasT_v[:, 0:NHF, 0:TQ])
                for tcol in range(1, NTCOL):
                    proj_col(tcol, wk_a, wk_b, kT_sb, do_v=True)

                # pass B reuses the wk ring slots (dep-ordered, load overlaps
                # the pass-A tail) and the same x ring
                wq_a = p1w.tile([128, KH, DPC], F32R, tag="wka")
                wq_b = p1w.tile([128, KH, DPC], F32R, tag="wkb")
                nc.sync.dma_start(out=wq_a[:], in_=wqT_v[:, 0:KH, :])
                nc.scalar.dma_start(out=wq_b[:], in_=wqT_v[:, KH:KC, :])
                for tcol in range(NTCOL):
                    proj_col(tcol, wq_a, wq_b, q_sb, do_v=False)

            if dbg:
                nc.sync.dma_start(out=kdump[:], in_=kT_sb[:])
                nc.sync.dma_start(out=qdump[:], in_=q_sb[:])
                nc.sync.dma_start(out=vdump[:], in_=v_sb[:])

            # -------- Phase 2: attention + output projection --------
            with (
                tc.tile_pool(name="p3wo", bufs=1) as p3wo,
                tc.tile_pool(name="p3b1", bufs=1) as p3b1,
                tc.tile_pool(name="p3s", bufs=2) as p3s,
                tc.tile_pool(name="p3er", bufs=2) as p3er,
                tc.tile_pool(name="p3ot", bufs=8) as p3ot,
                tc.tile_pool(name="p3recip", bufs=2) as p3recip,
                tc.tile_pool(name="p3out", bufs=2) as p3out,
                tc.tile_pool(name="psS", bufs=2, space="PSUM") as psS,
                tc.tile_pool(name="psAZ", bufs=2, space="PSUM") as psAZ,
                tc.tile_pool(name="psO", bufs=2, space="PSUM") as psO,
            ):
                wo_sb = p3wo.tile([128, HC, D], BF16)
                for hh in range(HC):
                    eng = nc.sync if hh % 2 == 0 else nc.scalar
                    eng.dma_start(out=wo_sb[:, hh, :], in_=woT_v[:, hh, :])

                for qt in range(NTQ):
                    q0 = qt * TQ
                    # bias for this query tile, both key halves
                    bias_h = [None, None]
                    for half in range(2):
                        if qt == 0 and half == 0:
                            bias_h[0] = bias0  # prefetched during pass A
                            continue
                        pool = p3b0 if half == 0 else p3b1
                        bc = pool.tile([128, NHF, TQ], F32, tag=f"b{half}")
                        nc.gpsimd.dma_start(
                            out=bc[:],
                            in_=biasT_v[:, half * NHF:(half + 1) * NHF,
                                        q0:q0 + TQ],
                        )
                        bias_h[half] = bc

                    ot_tiles = []
                    for h in range(HC):
                        qcol = q_sb[:, h, q0:q0 + TQ]
                        s_buf = p3s.tile([128, NTK, TQ], F32, tag="s")
                        # scores: stationary kT blocks, psum singles -> s_buf
                        for kb in range(NTK):
                            sps = psS.tile([128, TQ], F32, tag="sps")
                            nc.tensor.matmul(
                                sps[:],
                                kT_sb[:, h, kb * 128:(kb + 1) * 128],
                                qcol,
                                start=True,
                                stop=True,
                            )
                            bh = bias_h[kb // NHF]
                            nc.vector.tensor_add(
                                s_buf[:, kb, :],
                                sps[:],
                                bh[:, kb % NHF, :],
                            )
                        av = psAZ.tile([128, TQ], F32, tag="av")
                        zb = psAZ.tile([128, TQ], F32, tag="zb")
                        for half in range(2):
                            hs = slice(half * NHF, (half + 1) * NHF)
                            s_flat = s_buf[:, hs, :].rearrange(
                                "p a b -> p (a b)")
                            nc.scalar.activation(
                                s_flat, s_flat,
                                mybir.ActivationFunctionType.Tanh,
                                scale=1.0 / cap,
                            )
                            er = p3er.tile([128, NHF, TQ], BF16, tag="er")
                            nc.scalar.activation(
                                er[:].rearrange("p a b -> p (a b)"),
                                s_flat,
                                mybir.ActivationFunctionType.Exp,
                                scale=cap,
                            )
                            for kk in range(NHF):
                                kb = half * NHF + kk
                                nc.tensor.matmul(
                                    av[:],
                                    v_sb[:, kb, h * DK:(h + 1) * DK],
                                    er[:, kk, :],
                                    start=(kb == 0),
                                    stop=(kb == NTK - 1),
                                )
                                nc.tensor.matmul(
                                    zb[:],
                                    ones_full[:],
                                    er[:, kk, :],
                                    start=(kb == 0),
                                    stop=(kb == NTK - 1),
                                )
                        recip = p3recip.tile([128, TQ], F32, tag="recip")
                        nc.vector.reciprocal_approx_fast(
                            out=recip[:], in_=zb[:])
                        ot = p3ot.tile([128, TQ], BF16, tag="ot")
                        nc.vector.tensor_mul(ot[:], av[:], recip[:])
                        ot_tiles.append(ot)

                    # output projection for this query tile
                    for qs in range(TQ // 128):
                        for dc in range(4):
                            d0 = dc * 512
                            op = psO.tile([128, 512], F32, tag="op")
                            for h in range(HC):
                                nc.tensor.matmul(
                                    op[:],
                                    ot_tiles[h][:, qs * 128:(qs + 1) * 128],
                                    wo_sb[:, h, d0:d0 + 512],
                                    start=(h == 0),
                                    stop=(h == HC - 1),
                                )
                            outt = p3out.tile([128, 512], F32, tag="outt")
                            nc.vector.tensor_copy(outt[:], op[:])
                            nc.gpsimd.dma_start(
                                out=out_d[q0 + qs * 128:q0 + (qs + 1) * 128,
                                          d0:d0 + 512],
                                in_=outt[:],
                            )

            pbias.__exit__(None, None, None)

    nc.compile()
    return nc


_PROGRAM_CACHE: dict = {}


def _get_program(cap: float):
    if cap not in _PROGRAM_CACHE:
        _PROGRAM_CACHE[cap] = _build_program(cap)
    return _PROGRAM_CACHE[cap]


def _prepare_in_maps(inp, wq, wk, wv, wo, attn_bias, softcap):
    inp = np.asarray(inp, dtype=np.float32)
    xTs = [
        _round_fp32r(np.ascontiguousarray(inp[b].T)) for b in range(B)
    ]
    biasT = np.ascontiguousarray(
        np.asarray(attn_bias, dtype=np.float32).reshape(S, S).T
    )
    wq = np.asarray(wq, dtype=np.float32)
    wk = np.asarray(wk, dtype=np.float32)
    wv = np.asarray(wv, dtype=np.float32)
    wo = np.asarray(wo, dtype=np.float32)
    scale = 1.0 / np.sqrt(np.float32(DK))

    in_maps = []
    for c in range(NCORES):
        b = c // 4
        g = c % 4
        rows = slice(g * DPC, (g + 1) * DPC)
        in_maps.append({
            "xT": xTs[b],
            "ones": np.ones((128, 128), dtype=_bf16),
            "wqT": _round_fp32r((wq[rows] * scale).T),
            "wkT": _round_fp32r(wk[rows].T),
            "wvT": _round_fp32r(wv[rows].T),
            "woT": np.ascontiguousarray(wo[:, rows].T).astype(_bf16),
            "biasT": biasT,
        })
    return in_maps


def run(inputs: dict, trace: bool = False):
    """Run the SPMD kernel. Returns (full_output, BassKernelResults)."""
    cap = float(inputs["softcap"])
    nc = _get_program(cap)
    in_maps = _prepare_in_maps(
        inputs["inp"], inputs["wq"], inputs["wk"], inputs["wv"],
        inputs["wo"], inputs["attn_bias"], inputs["softcap"],
    )
    res = run_bass_kernel_spmd(
        nc, in_maps, list(range(NCORES)), trace=trace,
    )
    out = np.zeros((B, S, D), dtype=np.float64)
    for c in range(NCORES):
        out[c // 4] += res.results[c]["out_partial"]
    return out.astype(np.float32), res


def kernel(**inputs) -> np.ndarray:
    out, _ = run(inputs, trace=False)
    return out


if __name__ == "__main__":
    rng = np.random.default_rng(0)
    sc = 1.0 / np.sqrt(D)
    inputs = {
        "inp": rng.standard_normal((B, S, D)).astype(np.float32),
        "wq": (rng.standard_normal((D, D)) * sc).astype(np.float32),
        "wk": (rng.standard_normal((D, D)) * sc).astype(np.float32),
        "wv": (rng.standard_normal((D, D)) * sc).astype(np.float32),
        "wo": (rng.standard_normal((D, D)) * sc).astype(np.float32),
        "attn_bias": rng.standard_normal((1, 1, S, S)).astype(np.float32),
        "softcap": 30,
    }
    out = kernel(**inputs)
    print("out", out.shape, out.dtype, float(np.abs(out).max()))


# revision 45
# speedup vs baseline: 1.1600x; 1.1417x over previous
"""Multi-head self-attention with SDPA softcap, sharded over 8 NeuronCores.

Sharding: batch x head-group tensor parallel. Core c owns batch c//4 and
heads [4*(c%4), 4*(c%4)+4) (4 of 16 heads, 512 of 2048 dims):
  - projection pass: k,v,q projections for its batch in one sweep over x
    (bf16 weights/activations), all SBUF resident,
  - attention: softcap tanh + softmax per 512-query tile (scores accumulate
    in fp32 psum, bias added on DVE, exp on ACT; softmax denominator via a
    DVE pair-tree + a short ones-matmul),
  - output projection with its row-slice of wo -> partial [S, D] output.
Host sums the 4 partials per batch.
"""

import sys

if "/opt/trn_rl_repo" not in sys.path:
    sys.path.insert(0, "/opt/trn_rl_repo")

import numpy as np
import ml_dtypes

_bf16 = ml_dtypes.bfloat16

import concourse.bass as bass
import concourse.bacc as bacc
import concourse.tile as tile
from concourse import mybir
from concourse.bass_utils import run_bass_kernel_spmd

F32 = mybir.dt.float32
F32R = mybir.dt.float32r
BF16 = mybir.dt.bfloat16

D = 2048          # model dim
H = 16            # total heads
DK = 128          # head dim
B = 2
S = 2048
NCORES = 8
HC = 4            # heads per core
DPC = HC * DK     # 512: d' slice per core

KC = D // 128     # 16 contraction chunks over model dim
TCOL = 1024       # projection token-column width
NTCOL = S // TCOL             # 2
TQ = 512          # query-tile width
NTQ = S // TQ                 # 4
NTK = S // 128    # 16 key blocks
NHF = NTK // 2    # 8 key blocks per half
KH = KC // 2


def _build_program(cap: float, dbg: bool = False):
    nc = bacc.Bacc("TRN2", target_bir_lowering=False, debug=False,
                   num_devices=NCORES)

    xT = nc.dram_tensor("xT", [D, S], BF16, kind="ExternalInput").ap()
    ones_d = nc.dram_tensor("ones", [128, 128], BF16, kind="ExternalInput").ap()
    wqT = nc.dram_tensor("wqT", [D, DPC], BF16, kind="ExternalInput").ap()
    wkT = nc.dram_tensor("wkT", [D, DPC], BF16, kind="ExternalInput").ap()
    wvT = nc.dram_tensor("wvT", [D, DPC], BF16, kind="ExternalInput").ap()
    woT = nc.dram_tensor("woT", [DPC, D], BF16, kind="ExternalInput").ap()
    biasT = nc.dram_tensor("biasT", [S, S], BF16, kind="ExternalInput").ap()
    out_d = nc.dram_tensor("out_partial", [S, D], F32, kind="ExternalOutput").ap()
    if dbg:
        kdump = nc.dram_tensor("kdump", [128, HC, S], BF16, kind="ExternalOutput").ap()
        qdump = nc.dram_tensor("qdump", [128, HC, S], BF16, kind="ExternalOutput").ap()
        vdump = nc.dram_tensor("vdump", [128, NTK, DPC], BF16, kind="ExternalOutput").ap()

    xT_v = xT.rearrange("(kc p) t -> p kc t", p=128)
    biasT_v = biasT.rearrange("(kb p) q -> p kb q", p=128)
    wqT_v = wqT.rearrange("(kc p) n -> p kc n", p=128)
    wkT_v = wkT.rearrange("(kc p) n -> p kc n", p=128)
    wvT_v = wvT.rearrange("(kc p) n -> p kc n", p=128)
    woT_v = woT.rearrange("(h p) n -> p h n", p=128)

    with tile.TileContext(nc) as tc:
        with (
            tc.tile_pool(name="const", bufs=1) as cpool,
            tc.tile_pool(name="resid", bufs=1) as resid,
        ):
            ones_full = cpool.tile([128, 128], BF16)
            nc.sync.dma_start(out=ones_full[:], in_=ones_d[:])

            # SBUF-resident tensors (live through the whole kernel)
            kT_sb = resid.tile([128, HC, S], BF16)        # [dk, h, tok]
            v_sb = resid.tile([128, NTK, DPC], BF16)      # [tok%, tokb, (h dk)]
            q_sb = resid.tile([128, HC, S], BF16)         # [dk, h, tok]

            pbias = tc.tile_pool(name="p3b0", bufs=1)
            p3b0 = pbias.__enter__()
            bias0 = p3b0.tile([128, NHF, TQ], BF16, tag="b0")

            # ---------- Projection pass: k, v, q in one sweep over x -------
            with (
                tc.tile_pool(name="p1w", bufs=1) as p1w,
                tc.tile_pool(name="p1x", bufs=4) as p1x,
                tc.tile_pool(name="p1ps", bufs=2, space="PSUM") as p1ps,
                tc.tile_pool(name="p1psv", bufs=1, space="PSUM") as p1psv,
            ):
                wk_sb = p1w.tile([128, KC, DPC], BF16, tag="wk")
                wv_sb = p1w.tile([128, KC, DPC], BF16, tag="wv")
                wq_sb = p1w.tile([128, KC, DPC], BF16, tag="wq")
                nc.sync.dma_start(out=wk_sb[:], in_=wkT_v)
                nc.scalar.dma_start(out=wv_sb[:], in_=wvT_v)
                nc.scalar.dma_start(out=wq_sb[:], in_=wqT_v)

                for tcol in range(NTCOL):
                    t0 = tcol * TCOL
                    # x in quarter-column tiles: fine-grained ring frees let
                    # the next column's loads overlap this column's compute
                    xq = []
                    for j in range(4):
                        xt = p1x.tile([128, 4, TCOL], BF16, tag="x",
                                      name=f"xq{j}")
                        nc.gpsimd.dma_start(
                            out=xt[:], in_=xT_v[:, j * 4:(j + 1) * 4,
                                                t0:t0 + TCOL])
                        xq.append(xt)
                    # k then q: stationary weight chunks -> [dk, tok] layout
                    for w_sb, dst in ((wk_sb, kT_sb), (wq_sb, q_sb)):
                        for m in range(HC):
                            ps = p1ps.tile([128, TCOL], F32, tag="psk")
                            for kc in range(KC):
                                for ph in range(TCOL // 512):
                                    nc.tensor.matmul(
                                        ps[:, ph * 512:(ph + 1) * 512],
                                        w_sb[:, kc, m * 128:(m + 1) * 128],
                                        xq[kc // 4][:, kc % 4,
                                                    ph * 512:(ph + 1) * 512],
                                        start=(kc == 0),
                                        stop=(kc == KC - 1),
                                    )
                            nc.vector.tensor_copy(
                                dst[:, m, t0:t0 + TCOL], ps[:])
                    # v: stationary x chunks -> natural [tok, (h dk)] layout
                    for tsh in range(TCOL // 512):
                        vps = [p1psv.tile([128, DPC], F32, tag=f"psv{ts}",
                                          name=f"vp{ts}")
                               for ts in range(4)]
                        for kc in range(KC):
                            for ts4 in range(4):
                                ts = tsh * 4 + ts4
                                nc.tensor.matmul(
                                    vps[ts4][:],
                                    xq[kc // 4][:, kc % 4,
                                                ts * 128:(ts + 1) * 128],
                                    wv_sb[:, kc, :],
                                    start=(kc == 0),
                                    stop=(kc == KC - 1),
                                )
                        for ts4 in range(4):
                            nc.vector.tensor_copy(
                                v_sb[:, tcol * (TCOL // 128) + tsh * 4 + ts4,
                                     :],
                                vps[ts4][:])
                    if tcol == 0:
                        # prefetch first attention bias tile
                        nc.gpsimd.dma_start(
                            out=bias0[:], in_=biasT_v[:, 0:NHF, 0:TQ])

            if dbg:
                nc.sync.dma_start(out=kdump[:], in_=kT_sb[:])
                nc.sync.dma_start(out=qdump[:], in_=q_sb[:])
                nc.sync.dma_start(out=vdump[:], in_=v_sb[:])

            # -------- Phase 2: attention + output projection --------
            with (
                tc.tile_pool(name="p3wo", bufs=1) as p3wo,
                tc.tile_pool(name="p3b1", bufs=1) as p3b1,
                tc.tile_pool(name="p3s", bufs=2) as p3s,
                tc.tile_pool(name="p3er", bufs=2) as p3er,
                tc.tile_pool(name="p3et", bufs=2) as p3et,
                tc.tile_pool(name="p3ot", bufs=8) as p3ot,
                tc.tile_pool(name="p3recip", bufs=2) as p3recip,
                tc.tile_pool(name="p3out", bufs=2) as p3out,
                tc.tile_pool(name="psS", bufs=2, space="PSUM") as psS,
                tc.tile_pool(name="psAZ", bufs=2, space="PSUM") as psAZ,
                tc.tile_pool(name="psO", bufs=2, space="PSUM") as psO,
            ):
                wo_sb = p3wo.tile([128, HC, D], BF16)
                for hh in range(HC):
                    eng = nc.sync if hh % 2 == 0 else nc.scalar
                    eng.dma_start(out=wo_sb[:, hh, :], in_=woT_v[:, hh, :])

                for qt in range(NTQ):
                    q0 = qt * TQ
                    # bias for this query tile, both key halves
                    bias_h = [None, None]
                    for half in range(2):
                        if qt == 0 and half == 0:
                            bias_h[0] = bias0  # prefetched during pass A
                            continue
                        pool = p3b0 if half == 0 else p3b1
                        bc = pool.tile([128, NHF, TQ], BF16, tag=f"b{half}")
                        nc.gpsimd.dma_start(
                            out=bc[:],
                            in_=biasT_v[:, half * NHF:(half + 1) * NHF,
                                        q0:q0 + TQ],
                        )
                        bias_h[half] = bc

                    ot_tiles = []
                    for h in range(HC):
                        qcol = q_sb[:, h, q0:q0 + TQ]
                        s_buf = p3s.tile([128, NTK, TQ], F32, tag="s")
                        # scores: stationary kT blocks, psum singles -> s_buf
                        for kb in range(NTK):
                            sps = psS.tile([128, TQ], F32, tag="sps")
                            nc.tensor.matmul(
                                sps[:],
                                kT_sb[:, h, kb * 128:(kb + 1) * 128],
                                qcol,
                                start=True,
                                stop=True,
                            )
                            bh = bias_h[kb // NHF]
                            nc.vector.tensor_add(
                                s_buf[:, kb, :],
                                sps[:],
                                bh[:, kb % NHF, :],
                            )
                        av = psAZ.tile([128, TQ], F32, tag="av")
                        zb = psAZ.tile([128, TQ], F32, tag="zb")
                        for half in range(2):
                            hs = slice(half * NHF, (half + 1) * NHF)
                            s_flat = s_buf[:, hs, :].rearrange(
                                "p a b -> p (a b)")
                            nc.scalar.activation(
                                s_flat, s_flat,
                                mybir.ActivationFunctionType.Tanh,
                                scale=1.0 / cap,
                            )
                            er = p3er.tile([128, NHF, TQ], BF16, tag="er")
                            nc.scalar.activation(
                                er[:].rearrange("p a b -> p (a b)"),
                                s_flat,
                                mybir.ActivationFunctionType.Exp,
                                scale=cap,
                            )
                            for kk in range(NHF):
                                kb = half * NHF + kk
                                nc.tensor.matmul(
                                    av[:],
                                    v_sb[:, kb, h * DK:(h + 1) * DK],
                                    er[:, kk, :],
                                    start=(kb == 0),
                                    stop=(kb == NTK - 1),
                                )
                            # denominator pair-tree on DVE: 8 -> 2 blocks
                            er2 = p3et.tile([128, 4, TQ], BF16, tag="er2")
                            nc.vector.tensor_add(
                                er2[:], er[:, 0:NHF:2, :], er[:, 1:NHF:2, :])
                            er4 = p3et.tile([128, 2, TQ], BF16, tag="er4")
                            nc.vector.tensor_add(
                                er4[:], er2[:, 0:4:2, :], er2[:, 1:4:2, :])
                            for j in range(2):
                                nc.tensor.matmul(
                                    zb[:],
                                    ones_full[:],
                                    er4[:, j, :],
                                    start=(half == 0 and j == 0),
                                    stop=(half == 1 and j == 1),
                                )
                        recip = p3recip.tile([128, TQ], F32, tag="recip")
                        nc.vector.reciprocal_approx_fast(
                            out=recip[:], in_=zb[:])
                        ot = p3ot.tile([128, TQ], BF16, tag="ot")
                        nc.vector.tensor_mul(ot[:], av[:], recip[:])
                        ot_tiles.append(ot)

                    # output projection for this query tile
                    for qs in range(TQ // 128):
                        for dc in range(4):
                            d0 = dc * 512
                            op = psO.tile([128, 512], F32, tag="op")
                            for h in range(HC):
                                nc.tensor.matmul(
                                    op[:],
                                    ot_tiles[h][:, qs * 128:(qs + 1) * 128],
                                    wo_sb[:, h, d0:d0 + 512],
                                    start=(h == 0),
                                    stop=(h == HC - 1),
                                )
                            outt = p3out.tile([128, 512], F32, tag="outt")
                            nc.vector.tensor_copy(outt[:], op[:])
                            nc.gpsimd.dma_start(
                                out=out_d[q0 + qs * 128:q0 + (qs + 1) * 128,
                                          d0:d0 + 512],
                                in_=outt[:],
                            )

            pbias.__exit__(None, None, None)

    nc.compile()
    return nc


_PROGRAM_CACHE: dict = {}


def _get_program(cap: float):
    if cap not in _PROGRAM_CACHE:
        _PROGRAM_CACHE[cap] = _build_program(cap)
    return _PROGRAM_CACHE[cap]


def _prepare_in_maps(inp, wq, wk, wv, wo, attn_bias, softcap):
    inp = np.asarray(inp, dtype=np.float32)
    xTs = [
        np.ascontiguousarray(inp[b].T).astype(_bf16) for b in range(B)
    ]
    biasT = np.ascontiguousarray(
        np.asarray(attn_bias, dtype=np.float32).reshape(S, S).T
    ).astype(_bf16)
    wq = np.asarray(wq, dtype=np.float32)
    wk = np.asarray(wk, dtype=np.float32)
    wv = np.asarray(wv, dtype=np.float32)
    wo = np.asarray(wo, dtype=np.float32)
    scale = 1.0 / np.sqrt(np.float32(DK))

    in_maps = []
    for c in range(NCORES):
        b = c // 4
        g = c % 4
        rows = slice(g * DPC, (g + 1) * DPC)
        in_maps.append({
            "xT": xTs[b],
            "ones": np.ones((128, 128), dtype=_bf16),
            "wqT": np.ascontiguousarray((wq[rows] * scale).T).astype(_bf16),
            "wkT": np.ascontiguousarray(wk[rows].T).astype(_bf16),
            "wvT": np.ascontiguousarray(wv[rows].T).astype(_bf16),
            "woT": np.ascontiguousarray(wo[:, rows].T).astype(_bf16),
            "biasT": biasT,
        })
    return in_maps


def run(inputs: dict, trace: bool = False):
    """Run the SPMD kernel. Returns (full_output, BassKernelResults)."""
    cap = float(inputs["softcap"])
    nc = _get_program(cap)
    in_maps = _prepare_in_maps(
        inputs["inp"], inputs["wq"], inputs["wk"], inputs["wv"],
        inputs["wo"], inputs["attn_bias"], inputs["softcap"],
    )
    res = run_bass_kernel_spmd(
        nc, in_maps, list(range(NCORES)), trace=trace,
    )
    out = np.zeros((B, S, D), dtype=np.float64)
    for c in range(NCORES):
        out[c // 4] += res.results[c]["out_partial"]
    return out.astype(np.float32), res


def kernel(**inputs) -> np.ndarray:
    out, _ = run(inputs, trace=False)
    return out


if __name__ == "__main__":
    rng = np.random.default_rng(0)
    sc = 1.0 / np.sqrt(D)
    inputs = {
        "inp": rng.standard_normal((B, S, D)).astype(np.float32),
        "wq": (rng.standard_normal((D, D)) * sc).astype(np.float32),
        "wk": (rng.standard_normal((D, D)) * sc).astype(np.float32),
        "wv": (rng.standard_normal((D, D)) * sc).astype(np.float32),
        "wo": (rng.standard_normal((D, D)) * sc).astype(np.float32),
        "attn_bias": rng.standard_normal((1, 1, S, S)).astype(np.float32),
        "softcap": 30,
    }
    out = kernel(**inputs)
    print("out", out.shape, out.dtype, float(np.abs(out).max()))


# revision 48
# speedup vs baseline: 1.1677x; 1.0066x over previous
"""Multi-head self-attention with SDPA softcap, sharded over 8 NeuronCores.

Sharding: batch x head-group tensor parallel. Core c owns batch c//4 and
heads [4*(c%4), 4*(c%4)+4) (4 of 16 heads, 512 of 2048 dims):
  - projection pass: k,v,q projections for its batch in one sweep over x
    (bf16 weights/activations), all SBUF resident,
  - attention: softcap tanh + softmax per 512-query tile (scores accumulate
    in fp32 psum, bias added on DVE, exp on ACT; softmax denominator via a
    DVE pair-tree + a short ones-matmul),
  - output projection with its row-slice of wo -> partial [S, D] output.
Host sums the 4 partials per batch.
"""

import sys

if "/opt/trn_rl_repo" not in sys.path:
    sys.path.insert(0, "/opt/trn_rl_repo")

import numpy as np
import ml_dtypes

_bf16 = ml_dtypes.bfloat16

import concourse.bass as bass
import concourse.bacc as bacc
import concourse.tile as tile
from concourse import mybir
from concourse.bass_utils import run_bass_kernel_spmd

F32 = mybir.dt.float32
F32R = mybir.dt.float32r
BF16 = mybir.dt.bfloat16

D = 2048          # model dim
H = 16            # total heads
DK = 128          # head dim
B = 2
S = 2048
NCORES = 8
HC = 4            # heads per core
DPC = HC * DK     # 512: d' slice per core

KC = D // 128     # 16 contraction chunks over model dim
TCOL = 1024       # projection token-column width
NTCOL = S // TCOL             # 2
TQ = 512          # query-tile width
NTQ = S // TQ                 # 4
NTK = S // 128    # 16 key blocks
NHF = NTK // 2    # 8 key blocks per half
KH = KC // 2


def _build_program(cap: float, dbg: bool = False):
    nc = bacc.Bacc("TRN2", target_bir_lowering=False, debug=False,
                   num_devices=NCORES)

    xT = nc.dram_tensor("xT", [D, S], BF16, kind="ExternalInput").ap()
    ones_d = nc.dram_tensor("ones", [128, 128], BF16, kind="ExternalInput").ap()
    wqT = nc.dram_tensor("wqT", [D, DPC], BF16, kind="ExternalInput").ap()
    wkT = nc.dram_tensor("wkT", [D, DPC], BF16, kind="ExternalInput").ap()
    wvT = nc.dram_tensor("wvT", [D, DPC], BF16, kind="ExternalInput").ap()
    woT = nc.dram_tensor("woT", [DPC, D], BF16, kind="ExternalInput").ap()
    biasT = nc.dram_tensor("biasT", [S, S], BF16, kind="ExternalInput").ap()
    out_d = nc.dram_tensor("out_partial", [S, D], F32, kind="ExternalOutput").ap()
    if dbg:
        kdump = nc.dram_tensor("kdump", [128, HC, S], BF16, kind="ExternalOutput").ap()
        qdump = nc.dram_tensor("qdump", [128, HC, S], BF16, kind="ExternalOutput").ap()
        vdump = nc.dram_tensor("vdump", [128, NTK, DPC], BF16, kind="ExternalOutput").ap()

    xT_v = xT.rearrange("(kc p) t -> p kc t", p=128)
    biasT_v = biasT.rearrange("(kb p) q -> p kb q", p=128)
    wqT_v = wqT.rearrange("(kc p) n -> p kc n", p=128)
    wkT_v = wkT.rearrange("(kc p) n -> p kc n", p=128)
    wvT_v = wvT.rearrange("(kc p) n -> p kc n", p=128)
    woT_v = woT.rearrange("(h p) n -> p h n", p=128)

    with tile.TileContext(nc) as tc:
        with (
            tc.tile_pool(name="const", bufs=1) as cpool,
            tc.tile_pool(name="resid", bufs=1) as resid,
        ):
            ones_full = cpool.tile([128, 128], BF16)
            nc.sync.dma_start(out=ones_full[:], in_=ones_d[:])

            # SBUF-resident tensors (live through the whole kernel)
            kT_sb = resid.tile([128, HC, S], BF16)        # [dk, h, tok]
            v_sb = resid.tile([128, NTK, DPC], BF16)      # [tok%, tokb, (h dk)]
            q_sb = resid.tile([128, HC, S], BF16)         # [dk, h, tok]

            pbias = tc.tile_pool(name="p3b0", bufs=1)
            p3b0 = pbias.__enter__()
            bias0 = p3b0.tile([128, NHF, TQ], BF16, tag="b0")

            # ---------- Projection pass: k, v, q in one sweep over x -------
            with (
                tc.tile_pool(name="p1w", bufs=1) as p1w,
                tc.tile_pool(name="p1x", bufs=4) as p1x,
                tc.tile_pool(name="p1ps", bufs=2, space="PSUM") as p1ps,
                tc.tile_pool(name="p1psv", bufs=1, space="PSUM") as p1psv,
            ):
                wk_sb = p1w.tile([128, KC, DPC], BF16, tag="wk")
                wv_sb = p1w.tile([128, KC, DPC], BF16, tag="wv")
                wq_sb = p1w.tile([128, KC, DPC], BF16, tag="wq")
                for kq in range(4):
                    nc.sync.dma_start(out=wk_sb[:, kq * 4:(kq + 1) * 4, :],
                                      in_=wkT_v[:, kq * 4:(kq + 1) * 4, :])
                nc.scalar.dma_start(out=wv_sb[:], in_=wvT_v)
                nc.scalar.dma_start(out=wq_sb[:], in_=wqT_v)

                for tcol in range(NTCOL):
                    t0 = tcol * TCOL
                    # x in quarter-column tiles: fine-grained ring frees let
                    # the next column's loads overlap this column's compute
                    xq = []
                    for j in range(4):
                        xt = p1x.tile([128, 4, TCOL], BF16, tag="x",
                                      name=f"xq{j}")
                        nc.gpsimd.dma_start(
                            out=xt[:], in_=xT_v[:, j * 4:(j + 1) * 4,
                                                t0:t0 + TCOL])
                        xq.append(xt)
                    # k then q: stationary weight chunks -> [dk, tok] layout
                    for w_sb, dst in ((wk_sb, kT_sb), (wq_sb, q_sb)):
                        for m in range(HC):
                            ps = p1ps.tile([128, TCOL], F32, tag="psk")
                            for kc in range(KC):
                                for ph in range(TCOL // 512):
                                    nc.tensor.matmul(
                                        ps[:, ph * 512:(ph + 1) * 512],
                                        w_sb[:, kc, m * 128:(m + 1) * 128],
                                        xq[kc // 4][:, kc % 4,
                                                    ph * 512:(ph + 1) * 512],
                                        start=(kc == 0),
                                        stop=(kc == KC - 1),
                                    )
                            nc.vector.tensor_copy(
                                dst[:, m, t0:t0 + TCOL], ps[:])
                    # v: stationary x chunks -> natural [tok, (h dk)] layout
                    for tsh in range(TCOL // 512):
                        vps = [p1psv.tile([128, DPC], F32, tag=f"psv{ts}",
                                          name=f"vp{ts}")
                               for ts in range(4)]
                        for kc in range(KC):
                            for ts4 in range(4):
                                ts = tsh * 4 + ts4
                                nc.tensor.matmul(
                                    vps[ts4][:],
                                    xq[kc // 4][:, kc % 4,
                                                ts * 128:(ts + 1) * 128],
                                    wv_sb[:, kc, :],
                                    start=(kc == 0),
                                    stop=(kc == KC - 1),
                                )
                        for ts4 in range(4):
                            nc.vector.tensor_copy(
                                v_sb[:, tcol * (TCOL // 128) + tsh * 4 + ts4,
                                     :],
                                vps[ts4][:])
                    if tcol == 0:
                        # prefetch first attention bias tile
                        nc.gpsimd.dma_start(
                            out=bias0[:], in_=biasT_v[:, 0:NHF, 0:TQ])

            if dbg:
                nc.sync.dma_start(out=kdump[:], in_=kT_sb[:])
                nc.sync.dma_start(out=qdump[:], in_=q_sb[:])
                nc.sync.dma_start(out=vdump[:], in_=v_sb[:])

            # -------- Phase 2: attention + output projection --------
            with (
                tc.tile_pool(name="p3wo", bufs=1) as p3wo,
                tc.tile_pool(name="p3b1", bufs=1) as p3b1,
                tc.tile_pool(name="p3s", bufs=2) as p3s,
                tc.tile_pool(name="p3er", bufs=2) as p3er,
                tc.tile_pool(name="p3et", bufs=2) as p3et,
                tc.tile_pool(name="p3ot", bufs=8) as p3ot,
                tc.tile_pool(name="p3recip", bufs=2) as p3recip,
                tc.tile_pool(name="p3out", bufs=2) as p3out,
                tc.tile_pool(name="psS", bufs=2, space="PSUM") as psS,
                tc.tile_pool(name="psAZ", bufs=2, space="PSUM") as psAZ,
                tc.tile_pool(name="psO", bufs=2, space="PSUM") as psO,
            ):
                wo_sb = p3wo.tile([128, HC, D], BF16)
                for hh in range(HC):
                    eng = nc.sync if hh % 2 == 0 else nc.scalar
                    eng.dma_start(out=wo_sb[:, hh, :], in_=woT_v[:, hh, :])

                for qt in range(NTQ):
                    q0 = qt * TQ
                    # bias for this query tile, both key halves
                    bias_h = [None, None]
                    for half in range(2):
                        if qt == 0 and half == 0:
                            bias_h[0] = bias0  # prefetched during pass A
                            continue
                        pool = p3b0 if half == 0 else p3b1
                        bc = pool.tile([128, NHF, TQ], BF16, tag=f"b{half}")
                        nc.gpsimd.dma_start(
                            out=bc[:],
                            in_=biasT_v[:, half * NHF:(half + 1) * NHF,
                                        q0:q0 + TQ],
                        )
                        bias_h[half] = bc

                    ot_tiles = []
                    for h in range(HC):
                        qcol = q_sb[:, h, q0:q0 + TQ]
                        s_buf = p3s.tile([128, NTK, TQ], F32, tag="s")
                        # scores: stationary kT blocks, psum singles -> s_buf
                        for kb in range(NTK):
                            sps = psS.tile([128, TQ], F32, tag="sps")
                            nc.tensor.matmul(
                                sps[:],
                                kT_sb[:, h, kb * 128:(kb + 1) * 128],
                                qcol,
                                start=True,
                                stop=True,
                            )
                            bh = bias_h[kb // NHF]
                            nc.vector.tensor_add(
                                s_buf[:, kb, :],
                                sps[:],
                                bh[:, kb % NHF, :],
                            )
                        av = psAZ.tile([128, TQ], F32, tag="av")
                        zb = psAZ.tile([128, TQ], F32, tag="zb")
                        for half in range(2):
                            hs = slice(half * NHF, (half + 1) * NHF)
                            s_flat = s_buf[:, hs, :].rearrange(
                                "p a b -> p (a b)")
                            nc.scalar.activation(
                                s_flat, s_flat,
                                mybir.ActivationFunctionType.Tanh,
                                scale=1.0 / cap,
                            )
                            er = p3er.tile([128, NHF, TQ], BF16, tag="er")
                            nc.scalar.activation(
                                er[:].rearrange("p a b -> p (a b)"),
                                s_flat,
                                mybir.ActivationFunctionType.Exp,
                                scale=cap,
                            )
                            for kk in range(NHF):
                                kb = half * NHF + kk
                                nc.tensor.matmul(
                                    av[:],
                                    v_sb[:, kb, h * DK:(h + 1) * DK],
                                    er[:, kk, :],
                                    start=(kb == 0),
                                    stop=(kb == NTK - 1),
                                )
                            # denominator pair-tree on DVE: 8 -> 2 blocks
                            er2 = p3et.tile([128, 4, TQ], BF16, tag="er2")
                            nc.vector.tensor_add(
                                er2[:], er[:, 0:NHF:2, :], er[:, 1:NHF:2, :])
                            er4 = p3et.tile([128, 2, TQ], BF16, tag="er4")
                            nc.vector.tensor_add(
                                er4[:], er2[:, 0:4:2, :], er2[:, 1:4:2, :])
                            for j in range(2):
                                nc.tensor.matmul(
                                    zb[:],
                                    ones_full[:],
                                    er4[:, j, :],
                                    start=(half == 0 and j == 0),
                                    stop=(half == 1 and j == 1),
                                )
                        recip = p3recip.tile([128, TQ], F32, tag="recip")
                        nc.vector.reciprocal_approx_fast(
                            out=recip[:], in_=zb[:])
                        ot = p3ot.tile([128, TQ], BF16, tag="ot")
                        nc.vector.tensor_mul(ot[:], av[:], recip[:])
                        ot_tiles.append(ot)

                    # output projection for this query tile
                    for qs in range(TQ // 128):
                        for dc in range(4):
                            d0 = dc * 512
                            op = psO.tile([128, 512], F32, tag="op")
                            for h in range(HC):
                                nc.tensor.matmul(
                                    op[:],
                                    ot_tiles[h][:, qs * 128:(qs + 1) * 128],
                                    wo_sb[:, h, d0:d0 + 512],
                                    start=(h == 0),
                                    stop=(h == HC - 1),
                                )
                            outt = p3out.tile([128, 512], F32, tag="outt")
                            nc.vector.tensor_copy(outt[:], op[:])
                            nc.sync.dma_start(
                                out=out_d[q0 + qs * 128:q0 + (qs + 1) * 128,
                                          d0:d0 + 512],
                                in_=outt[:],
                            )

            pbias.__exit__(None, None, None)

    nc.compile()
    return nc


_PROGRAM_CACHE: dict = {}


def _get_program(cap: float):
    if cap not in _PROGRAM_CACHE:
        _PROGRAM_CACHE[cap] = _build_program(cap)
    return _PROGRAM_CACHE[cap]


def _prepare_in_maps(inp, wq, wk, wv, wo, attn_bias, softcap):
    inp = np.asarray(inp, dtype=np.float32)
    xTs = [
        np.ascontiguousarray(inp[b].T).astype(_bf16) for b in range(B)
    ]
    biasT = np.ascontiguousarray(
        np.asarray(attn_bias, dtype=np.float32).reshape(S, S).T
    ).astype(_bf16)
    wq = np.asarray(wq, dtype=np.float32)
    wk = np.asarray(wk, dtype=np.float32)
    wv = np.asarray(wv, dtype=np.float32)
    wo = np.asarray(wo, dtype=np.float32)
    scale = 1.0 / np.sqrt(np.float32(DK))

    in_maps = []
    for c in range(NCORES):
        b = c // 4
        g = c % 4
        rows = slice(g * DPC, (g + 1) * DPC)
        in_maps.append({
            "xT": xTs[b],
            "ones": np.ones((128, 128), dtype=_bf16),
            "wqT": np.ascontiguousarray((wq[rows] * scale).T).astype(_bf16),
            "wkT": np.ascontiguousarray(wk[rows].T).astype(_bf16),
            "wvT": np.ascontiguousarray(wv[rows].T).astype(_bf16),
            "woT": np.ascontiguousarray(wo[:, rows].T).astype(_bf16),
            "biasT": biasT,
        })
    return in_maps


def run(inputs: dict, trace: bool = False):
    """Run the SPMD kernel. Returns (full_output, BassKernelResults)."""
    cap = float(inputs["softcap"])
    nc = _get_program(cap)
    in_maps = _prepare_in_maps(
        inputs["inp"], inputs["wq"], inputs["wk"], inputs["wv"],
        inputs["wo"], inputs["attn_bias"], inputs["softcap"],
    )
    res = run_bass_kernel_spmd(
        nc, in_maps, list(range(NCORES)), trace=trace,
    )
    out = np.zeros((B, S, D), dtype=np.float64)
    for c in range(NCORES):
        out[c // 4] += res.results[c]["out_partial"]
    return out.astype(np.float32), res


def kernel(**inputs) -> np.ndarray:
    out, _ = run(inputs, trace=False)
    return out


if __name__ == "__main__":
    rng = np.random.default_rng(0)
    sc = 1.0 / np.sqrt(D)
    inputs = {
        "inp": rng.standard_normal((B, S, D)).astype(np.float32),
        "wq": (rng.standard_normal((D, D)) * sc).astype(np.float32),
        "wk": (rng.standard_normal((D, D)) * sc).astype(np.float32),
        "wv": (rng.standard_normal((D, D)) * sc).astype(np.float32),
        "wo": (rng.standard_normal((D, D)) * sc).astype(np.float32),
        "attn_bias": rng.standard_normal((1, 1, S, S)).astype(np.float32),
        "softcap": 30,
    }
    out = kernel(**inputs)
    print("out", out.shape, out.dtype, float(np.abs(out).max()))


# revision 49
# speedup vs baseline: 1.1768x; 1.0078x over previous
"""Multi-head self-attention with SDPA softcap, sharded over 8 NeuronCores.

Sharding: batch x head-group tensor parallel. Core c owns batch c//4 and
heads [4*(c%4), 4*(c%4)+4) (4 of 16 heads, 512 of 2048 dims):
  - projection pass: k,v,q projections for its batch in one sweep over x
    (bf16 weights/activations), all SBUF resident,
  - attention: softcap tanh + softmax per 512-query tile (scores accumulate
    in fp32 psum, bias added on DVE, exp on ACT; softmax denominator via a
    DVE pair-tree + a short ones-matmul),
  - output projection with its row-slice of wo -> partial [S, D] output.
Host sums the 4 partials per batch.
"""

import sys

if "/opt/trn_rl_repo" not in sys.path:
    sys.path.insert(0, "/opt/trn_rl_repo")

import numpy as np
import ml_dtypes

_bf16 = ml_dtypes.bfloat16

import concourse.bass as bass
import concourse.bacc as bacc
import concourse.tile as tile
from concourse import mybir
from concourse.bass_utils import run_bass_kernel_spmd

F32 = mybir.dt.float32
F32R = mybir.dt.float32r
BF16 = mybir.dt.bfloat16

D = 2048          # model dim
H = 16            # total heads
DK = 128          # head dim
B = 2
S = 2048
NCORES = 8
HC = 4            # heads per core
DPC = HC * DK     # 512: d' slice per core

KC = D // 128     # 16 contraction chunks over model dim
TCOL = 1024       # projection token-column width
NTCOL = S // TCOL             # 2
TQ = 512          # query-tile width
NTQ = S // TQ                 # 4
NTK = S // 128    # 16 key blocks
NHF = NTK // 2    # 8 key blocks per half
KH = KC // 2


def _build_program(cap: float, dbg: bool = False):
    nc = bacc.Bacc("TRN2", target_bir_lowering=False, debug=False,
                   num_devices=NCORES)

    xT = nc.dram_tensor("xT", [D, S], BF16, kind="ExternalInput").ap()
    ones_d = nc.dram_tensor("ones", [128, 128], BF16, kind="ExternalInput").ap()
    wqT = nc.dram_tensor("wqT", [D, DPC], BF16, kind="ExternalInput").ap()
    wkT = nc.dram_tensor("wkT", [D, DPC], BF16, kind="ExternalInput").ap()
    wvT = nc.dram_tensor("wvT", [D, DPC], BF16, kind="ExternalInput").ap()
    woT = nc.dram_tensor("woT", [DPC, D], BF16, kind="ExternalInput").ap()
    biasT = nc.dram_tensor("biasT", [S, S], BF16, kind="ExternalInput").ap()
    out_d = nc.dram_tensor("out_partial", [S, D], F32, kind="ExternalOutput").ap()
    if dbg:
        kdump = nc.dram_tensor("kdump", [128, HC, S], BF16, kind="ExternalOutput").ap()
        qdump = nc.dram_tensor("qdump", [128, HC, S], BF16, kind="ExternalOutput").ap()
        vdump = nc.dram_tensor("vdump", [128, NTK, DPC], BF16, kind="ExternalOutput").ap()

    xT_v = xT.rearrange("(kc p) t -> p kc t", p=128)
    biasT_v = biasT.rearrange("(kb p) q -> p kb q", p=128)
    wqT_v = wqT.rearrange("(kc p) n -> p kc n", p=128)
    wkT_v = wkT.rearrange("(kc p) n -> p kc n", p=128)
    wvT_v = wvT.rearrange("(kc p) n -> p kc n", p=128)
    woT_v = woT.rearrange("(h p) n -> p h n", p=128)

    with tile.TileContext(nc) as tc:
        with (
            tc.tile_pool(name="const", bufs=1) as cpool,
            tc.tile_pool(name="resid", bufs=1) as resid,
        ):
            ones_full = cpool.tile([128, 128], BF16)
            nc.sync.dma_start(out=ones_full[:], in_=ones_d[:])

            # SBUF-resident tensors (live through the whole kernel)
            kT_sb = resid.tile([128, HC, S], BF16)        # [dk, h, tok]
            v_sb = resid.tile([128, NTK, DPC], BF16)      # [tok%, tokb, (h dk)]
            q_sb = resid.tile([128, HC, S], BF16)         # [dk, h, tok]

            pbias = tc.tile_pool(name="p3b0", bufs=1)
            p3b0 = pbias.__enter__()
            bias0 = p3b0.tile([128, NHF, TQ], BF16, tag="b0")

            # ---------- Projection pass: k, v, q in one sweep over x -------
            with (
                tc.tile_pool(name="p1w", bufs=1) as p1w,
                tc.tile_pool(name="p1x", bufs=4) as p1x,
                tc.tile_pool(name="p1ps", bufs=2, space="PSUM") as p1ps,
                tc.tile_pool(name="p1psv", bufs=1, space="PSUM") as p1psv,
            ):
                wk_sb = p1w.tile([128, KC, DPC], BF16, tag="wk")
                wv_sb = p1w.tile([128, KC, DPC], BF16, tag="wv")
                wq_sb = p1w.tile([128, KC, DPC], BF16, tag="wq")
                for kq in range(4):
                    nc.sync.dma_start(out=wk_sb[:, kq * 4:(kq + 1) * 4, :],
                                      in_=wkT_v[:, kq * 4:(kq + 1) * 4, :])
                nc.scalar.dma_start(out=wv_sb[:], in_=wvT_v)
                nc.scalar.dma_start(out=wq_sb[:], in_=wqT_v)

                for tcol in range(NTCOL):
                    t0 = tcol * TCOL
                    # x in quarter-column tiles: fine-grained ring frees let
                    # the next column's loads overlap this column's compute
                    xq = []
                    for j in range(4):
                        xt = p1x.tile([128, 4, TCOL], BF16, tag="x",
                                      name=f"xq{j}")
                        nc.gpsimd.dma_start(
                            out=xt[:], in_=xT_v[:, j * 4:(j + 1) * 4,
                                                t0:t0 + TCOL])
                        xq.append(xt)
                    # k then q: stationary weight chunks -> [dk, tok] layout
                    for w_sb, dst in ((wk_sb, kT_sb), (wq_sb, q_sb)):
                        for m in range(HC):
                            ps = p1ps.tile([128, TCOL], F32, tag="psk")
                            for kc in range(KC):
                                for ph in range(TCOL // 512):
                                    nc.tensor.matmul(
                                        ps[:, ph * 512:(ph + 1) * 512],
                                        w_sb[:, kc, m * 128:(m + 1) * 128],
                                        xq[kc // 4][:, kc % 4,
                                                    ph * 512:(ph + 1) * 512],
                                        start=(kc == 0),
                                        stop=(kc == KC - 1),
                                    )
                            nc.vector.tensor_copy(
                                dst[:, m, t0:t0 + TCOL], ps[:])
                    # v: stationary x chunks -> natural [tok, (h dk)] layout
                    for tsh in range(TCOL // 512):
                        vps = [p1psv.tile([128, DPC], F32, tag=f"psv{ts}",
                                          name=f"vp{ts}")
                               for ts in range(4)]
                        for kc in range(KC):
                            for ts4 in range(4):
                                ts = tsh * 4 + ts4
                                nc.tensor.matmul(
                                    vps[ts4][:],
                                    xq[kc // 4][:, kc % 4,
                                                ts * 128:(ts + 1) * 128],
                                    wv_sb[:, kc, :],
                                    start=(kc == 0),
                                    stop=(kc == KC - 1),
                                )
                        for ts4 in range(4):
                            nc.vector.tensor_copy(
                                v_sb[:, tcol * (TCOL // 128) + tsh * 4 + ts4,
                                     :],
                                vps[ts4][:])
                    if tcol == 0:
                        # prefetch first attention bias tile
                        nc.gpsimd.dma_start(
                            out=bias0[:], in_=biasT_v[:, 0:NHF, 0:TQ])

            if dbg:
                nc.sync.dma_start(out=kdump[:], in_=kT_sb[:])
                nc.sync.dma_start(out=qdump[:], in_=q_sb[:])
                nc.sync.dma_start(out=vdump[:], in_=v_sb[:])

            # -------- Phase 2: attention + output projection --------
            with (
                tc.tile_pool(name="p3wo", bufs=1) as p3wo,
                tc.tile_pool(name="p3b1", bufs=1) as p3b1,
                tc.tile_pool(name="p3s", bufs=2) as p3s,
                tc.tile_pool(name="p3er", bufs=2) as p3er,
                tc.tile_pool(name="p3et", bufs=2) as p3et,
                tc.tile_pool(name="p3ot", bufs=8) as p3ot,
                tc.tile_pool(name="p3recip", bufs=2) as p3recip,
                tc.tile_pool(name="p3out", bufs=2) as p3out,
                tc.tile_pool(name="psS", bufs=2, space="PSUM") as psS,
                tc.tile_pool(name="psAZ", bufs=2, space="PSUM") as psAZ,
                tc.tile_pool(name="psO", bufs=2, space="PSUM") as psO,
            ):
                wo_sb = p3wo.tile([128, HC, D], BF16)
                for hh in range(HC):
                    eng = nc.sync if hh % 2 == 0 else nc.scalar
                    eng.dma_start(out=wo_sb[:, hh, :], in_=woT_v[:, hh, :])

                pending = []

                def emit_outproj(q0o, qs, ots):
                    for dc in range(4):
                        d0 = dc * 512
                        op = psO.tile([128, 512], F32, tag="op", name="op")
                        for h in range(HC):
                            nc.tensor.matmul(
                                op[:],
                                ots[h][:, qs * 128:(qs + 1) * 128],
                                wo_sb[:, h, d0:d0 + 512],
                                start=(h == 0),
                                stop=(h == HC - 1),
                            )
                        outt = p3out.tile([128, 512], F32, tag="outt",
                                          name="outt")
                        nc.vector.tensor_copy(outt[:], op[:])
                        nc.sync.dma_start(
                            out=out_d[q0o + qs * 128:q0o + (qs + 1) * 128,
                                      d0:d0 + 512],
                            in_=outt[:],
                        )

                for qt in range(NTQ):
                    q0 = qt * TQ
                    # bias for this query tile, both key halves
                    bias_h = [None, None]
                    for half in range(2):
                        if qt == 0 and half == 0:
                            bias_h[0] = bias0  # prefetched during pass A
                            continue
                        pool = p3b0 if half == 0 else p3b1
                        bc = pool.tile([128, NHF, TQ], BF16, tag=f"b{half}")
                        nc.gpsimd.dma_start(
                            out=bc[:],
                            in_=biasT_v[:, half * NHF:(half + 1) * NHF,
                                        q0:q0 + TQ],
                        )
                        bias_h[half] = bc

                    ot_tiles = []
                    for h in range(HC):
                        if pending:
                            emit_outproj(*pending.pop(0))
                        qcol = q_sb[:, h, q0:q0 + TQ]
                        s_buf = p3s.tile([128, NTK, TQ], F32, tag="s")
                        # scores: stationary kT blocks, psum singles -> s_buf
                        for kb in range(NTK):
                            sps = psS.tile([128, TQ], F32, tag="sps")
                            nc.tensor.matmul(
                                sps[:],
                                kT_sb[:, h, kb * 128:(kb + 1) * 128],
                                qcol,
                                start=True,
                                stop=True,
                            )
                            bh = bias_h[kb // NHF]
                            nc.vector.tensor_add(
                                s_buf[:, kb, :],
                                sps[:],
                                bh[:, kb % NHF, :],
                            )
                        av = psAZ.tile([128, TQ], F32, tag="av")
                        zb = psAZ.tile([128, TQ], F32, tag="zb")
                        for half in range(2):
                            hs = slice(half * NHF, (half + 1) * NHF)
                            s_flat = s_buf[:, hs, :].rearrange(
                                "p a b -> p (a b)")
                            nc.scalar.activation(
                                s_flat, s_flat,
                                mybir.ActivationFunctionType.Tanh,
                                scale=1.0 / cap,
                            )
                            er = p3er.tile([128, NHF, TQ], BF16, tag="er")
                            nc.scalar.activation(
                                er[:].rearrange("p a b -> p (a b)"),
                                s_flat,
                                mybir.ActivationFunctionType.Exp,
                                scale=cap,
                            )
                            for kk in range(NHF):
                                kb = half * NHF + kk
                                nc.tensor.matmul(
                                    av[:],
                                    v_sb[:, kb, h * DK:(h + 1) * DK],
                                    er[:, kk, :],
                                    start=(kb == 0),
                                    stop=(kb == NTK - 1),
                                )
                            # denominator pair-tree on DVE: 8 -> 2 blocks
                            er2 = p3et.tile([128, 4, TQ], BF16, tag="er2")
                            nc.vector.tensor_add(
                                er2[:], er[:, 0:NHF:2, :], er[:, 1:NHF:2, :])
                            er4 = p3et.tile([128, 2, TQ], BF16, tag="er4")
                            nc.vector.tensor_add(
                                er4[:], er2[:, 0:4:2, :], er2[:, 1:4:2, :])
                            for j in range(2):
                                nc.tensor.matmul(
                                    zb[:],
                                    ones_full[:],
                                    er4[:, j, :],
                                    start=(half == 0 and j == 0),
                                    stop=(half == 1 and j == 1),
                                )
                        recip = p3recip.tile([128, TQ], F32, tag="recip")
                        nc.vector.reciprocal_approx_fast(
                            out=recip[:], in_=zb[:])
                        ot = p3ot.tile([128, TQ], BF16, tag="ot")
                        nc.vector.tensor_mul(ot[:], av[:], recip[:])
                        ot_tiles.append(ot)

                    # defer output projection; interleave with next tile
                    for qs in range(TQ // 128):
                        pending.append((q0, qs, ot_tiles))
                for item in pending:
                    emit_outproj(*item)

            pbias.__exit__(None, None, None)

    nc.compile()
    return nc


_PROGRAM_CACHE: dict = {}


def _get_program(cap: float):
    if cap not in _PROGRAM_CACHE:
        _PROGRAM_CACHE[cap] = _build_program(cap)
    return _PROGRAM_CACHE[cap]


def _prepare_in_maps(inp, wq, wk, wv, wo, attn_bias, softcap):
    inp = np.asarray(inp, dtype=np.float32)
    xTs = [
        np.ascontiguousarray(inp[b].T).astype(_bf16) for b in range(B)
    ]
    biasT = np.ascontiguousarray(
        np.asarray(attn_bias, dtype=np.float32).reshape(S, S).T
    ).astype(_bf16)
    wq = np.asarray(wq, dtype=np.float32)
    wk = np.asarray(wk, dtype=np.float32)
    wv = np.asarray(wv, dtype=np.float32)
    wo = np.asarray(wo, dtype=np.float32)
    scale = 1.0 / np.sqrt(np.float32(DK))

    in_maps = []
    for c in range(NCORES):
        b = c // 4
        g = c % 4
        rows = slice(g * DPC, (g + 1) * DPC)
        in_maps.append({
            "xT": xTs[b],
            "ones": np.ones((128, 128), dtype=_bf16),
            "wqT": np.ascontiguousarray((wq[rows] * scale).T).astype(_bf16),
            "wkT": np.ascontiguousarray(wk[rows].T).astype(_bf16),
            "wvT": np.ascontiguousarray(wv[rows].T).astype(_bf16),
            "woT": np.ascontiguousarray(wo[:, rows].T).astype(_bf16),
            "biasT": biasT,
        })
    return in_maps


def run(inputs: dict, trace: bool = False):
    """Run the SPMD kernel. Returns (full_output, BassKernelResults)."""
    cap = float(inputs["softcap"])
    nc = _get_program(cap)
    in_maps = _prepare_in_maps(
        inputs["inp"], inputs["wq"], inputs["wk"], inputs["wv"],
        inputs["wo"], inputs["attn_bias"], inputs["softcap"],
    )
    res = run_bass_kernel_spmd(
        nc, in_maps, list(range(NCORES)), trace=trace,
    )
    out = np.zeros((B, S, D), dtype=np.float64)
    for c in range(NCORES):
        out[c // 4] += res.results[c]["out_partial"]
    return out.astype(np.float32), res


def kernel(**inputs) -> np.ndarray:
    out, _ = run(inputs, trace=False)
    return out


if __name__ == "__main__":
    rng = np.random.default_rng(0)
    sc = 1.0 / np.sqrt(D)
    inputs = {
        "inp": rng.standard_normal((B, S, D)).astype(np.float32),
        "wq": (rng.standard_normal((D, D)) * sc).astype(np.float32),
        "wk": (rng.standard_normal((D, D)) * sc).astype(np.float32),
        "wv": (rng.standard_normal((D, D)) * sc).astype(np.float32),
        "wo": (rng.standard_normal((D, D)) * sc).astype(np.float32),
        "attn_bias": rng.standard_normal((1, 1, S, S)).astype(np.float32),
        "softcap": 30,
    }
    out = kernel(**inputs)
    print("out", out.shape, out.dtype, float(np.abs(out).max()))


# revision 50
# speedup vs baseline: 1.2187x; 1.0356x over previous
"""Multi-head self-attention with SDPA softcap, sharded over 8 NeuronCores.

Sharding: batch x head-group tensor parallel. Core c owns batch c//4 and
heads [4*(c%4), 4*(c%4)+4) (4 of 16 heads, 512 of 2048 dims):
  - projection pass: k,v,q projections for its batch in one sweep over x
    (bf16 weights/activations), all SBUF resident,
  - attention: softcap tanh + softmax per 512-query tile (scores accumulate
    in fp32 psum, bias added on DVE, exp on ACT; softmax denominator via a
    DVE pair-tree + a short ones-matmul),
  - output projection with its row-slice of wo -> partial [S, D] output.
Host sums the 4 partials per batch.
"""

import sys

if "/opt/trn_rl_repo" not in sys.path:
    sys.path.insert(0, "/opt/trn_rl_repo")

import numpy as np
import ml_dtypes

_bf16 = ml_dtypes.bfloat16

import concourse.bass as bass
import concourse.bacc as bacc
import concourse.tile as tile
from concourse import mybir
from concourse.bass_utils import run_bass_kernel_spmd

F32 = mybir.dt.float32
F32R = mybir.dt.float32r
BF16 = mybir.dt.bfloat16

D = 2048          # model dim
H = 16            # total heads
DK = 128          # head dim
B = 2
S = 2048
NCORES = 8
HC = 4            # heads per core
DPC = HC * DK     # 512: d' slice per core

KC = D // 128     # 16 contraction chunks over model dim
TCOL = 1024       # projection token-column width
NTCOL = S // TCOL             # 2
TQ = 512          # query-tile width
NTQ = S // TQ                 # 4
NTK = S // 128    # 16 key blocks
NHF = NTK // 2    # 8 key blocks per half
KH = KC // 2


def _build_program(cap: float, dbg: bool = False):
    nc = bacc.Bacc("TRN2", target_bir_lowering=False, debug=False,
                   num_devices=NCORES)

    xT = nc.dram_tensor("xT", [D, S], BF16, kind="ExternalInput").ap()
    ones_d = nc.dram_tensor("ones", [128, 128], BF16, kind="ExternalInput").ap()
    wqT = nc.dram_tensor("wqT", [D, DPC], BF16, kind="ExternalInput").ap()
    wkT = nc.dram_tensor("wkT", [D, DPC], BF16, kind="ExternalInput").ap()
    wvT = nc.dram_tensor("wvT", [D, DPC], BF16, kind="ExternalInput").ap()
    woT = nc.dram_tensor("woT", [DPC, D], BF16, kind="ExternalInput").ap()
    biasT = nc.dram_tensor("biasT", [S, S], BF16, kind="ExternalInput").ap()
    out_d = nc.dram_tensor("out_partial", [S, D], F32, kind="ExternalOutput").ap()
    if dbg:
        kdump = nc.dram_tensor("kdump", [128, HC, S], BF16, kind="ExternalOutput").ap()
        qdump = nc.dram_tensor("qdump", [128, HC, S], BF16, kind="ExternalOutput").ap()
        vdump = nc.dram_tensor("vdump", [128, NTK, DPC], BF16, kind="ExternalOutput").ap()

    xT_v = xT.rearrange("(kc p) t -> p kc t", p=128)
    biasT_v = biasT.rearrange("(kb p) q -> p kb q", p=128)
    wqT_v = wqT.rearrange("(kc p) n -> p kc n", p=128)
    wkT_v = wkT.rearrange("(kc p) n -> p kc n", p=128)
    wvT_v = wvT.rearrange("(kc p) n -> p kc n", p=128)
    woT_v = woT.rearrange("(h p) n -> p h n", p=128)

    with tile.TileContext(nc) as tc:
        with (
            tc.tile_pool(name="const", bufs=1) as cpool,
            tc.tile_pool(name="resid", bufs=1) as resid,
        ):
            ones_full = cpool.tile([128, 128], BF16)
            nc.sync.dma_start(out=ones_full[:], in_=ones_d[:])

            # SBUF-resident tensors (live through the whole kernel)
            kT_sb = resid.tile([128, HC, S], BF16)        # [dk, h, tok]
            v_sb = resid.tile([128, NTK, DPC], BF16)      # [tok%, tokb, (h dk)]
            q_sb = resid.tile([128, HC, S], BF16)         # [dk, h, tok]

            pbias = tc.tile_pool(name="p3b0", bufs=1)
            p3b0 = pbias.__enter__()
            bias0 = p3b0.tile([128, NHF, TQ], BF16, tag="b0")

            # ---------- Projection pass: k, v, q in one sweep over x -------
            with (
                tc.tile_pool(name="p1w", bufs=1) as p1w,
                tc.tile_pool(name="p1x", bufs=4) as p1x,
                tc.tile_pool(name="p1ps", bufs=2, space="PSUM") as p1ps,
                tc.tile_pool(name="p1psv", bufs=1, space="PSUM") as p1psv,
            ):
                wk_sb = p1w.tile([128, KC, DPC], BF16, tag="wk")
                wv_sb = p1w.tile([128, KC, DPC], BF16, tag="wv")
                wq_sb = p1w.tile([128, KC, DPC], BF16, tag="wq")
                for kq in range(4):
                    nc.sync.dma_start(out=wk_sb[:, kq * 4:(kq + 1) * 4, :],
                                      in_=wkT_v[:, kq * 4:(kq + 1) * 4, :])
                nc.scalar.dma_start(out=wv_sb[:], in_=wvT_v)
                nc.scalar.dma_start(out=wq_sb[:], in_=wqT_v)

                for tcol in range(NTCOL):
                    t0 = tcol * TCOL
                    # x in quarter-column tiles: fine-grained ring frees let
                    # the next column's loads overlap this column's compute
                    xq = []
                    for j in range(4):
                        xt = p1x.tile([128, 4, TCOL], BF16, tag="x",
                                      name=f"xq{j}")
                        nc.gpsimd.dma_start(
                            out=xt[:], in_=xT_v[:, j * 4:(j + 1) * 4,
                                                t0:t0 + TCOL])
                        xq.append(xt)
                    # k then q: stationary weight chunks -> [dk, tok] layout
                    for w_sb, dst in ((wk_sb, kT_sb), (wq_sb, q_sb)):
                        for m in range(HC):
                            ps = p1ps.tile([128, TCOL], F32, tag="psk")
                            for kc in range(KC):
                                for ph in range(TCOL // 512):
                                    nc.tensor.matmul(
                                        ps[:, ph * 512:(ph + 1) * 512],
                                        w_sb[:, kc, m * 128:(m + 1) * 128],
                                        xq[kc // 4][:, kc % 4,
                                                    ph * 512:(ph + 1) * 512],
                                        start=(kc == 0),
                                        stop=(kc == KC - 1),
                                    )
                            nc.vector.tensor_copy(
                                dst[:, m, t0:t0 + TCOL], ps[:])
                    # v: stationary x chunks -> natural [tok, (h dk)] layout
                    for tsh in range(TCOL // 512):
                        vps = [p1psv.tile([128, DPC], F32, tag=f"psv{ts}",
                                          name=f"vp{ts}")
                               for ts in range(4)]
                        for kc in range(KC):
                            for ts4 in range(4):
                                ts = tsh * 4 + ts4
                                nc.tensor.matmul(
                                    vps[ts4][:],
                                    xq[kc // 4][:, kc % 4,
                                                ts * 128:(ts + 1) * 128],
                                    wv_sb[:, kc, :],
                                    start=(kc == 0),
                                    stop=(kc == KC - 1),
                                )
                        for ts4 in range(4):
                            nc.vector.tensor_copy(
                                v_sb[:, tcol * (TCOL // 128) + tsh * 4 + ts4,
                                     :],
                                vps[ts4][:])
                    if tcol == 0:
                        # prefetch first attention bias tile
                        nc.gpsimd.dma_start(
                            out=bias0[:], in_=biasT_v[:, 0:NHF, 0:TQ])

            if dbg:
                nc.sync.dma_start(out=kdump[:], in_=kT_sb[:])
                nc.sync.dma_start(out=qdump[:], in_=q_sb[:])
                nc.sync.dma_start(out=vdump[:], in_=v_sb[:])

            # -------- Phase 2: attention + output projection --------
            with (
                tc.tile_pool(name="p3wo", bufs=1) as p3wo,
                tc.tile_pool(name="p3b1", bufs=1) as p3b1,
                tc.tile_pool(name="p3s", bufs=2) as p3s,
                tc.tile_pool(name="p3er", bufs=2) as p3er,
                tc.tile_pool(name="p3et", bufs=2) as p3et,
                tc.tile_pool(name="p3ot", bufs=8) as p3ot,
                tc.tile_pool(name="p3recip", bufs=2) as p3recip,
                tc.tile_pool(name="p3out", bufs=2) as p3out,
                tc.tile_pool(name="psS", bufs=4, space="PSUM") as psS,
                tc.tile_pool(name="psAZ", bufs=1, space="PSUM") as psAZ,
                tc.tile_pool(name="psO", bufs=2, space="PSUM") as psO,
            ):
                wo_sb = p3wo.tile([128, HC, D], BF16)
                for hh in range(HC):
                    eng = nc.sync if hh % 2 == 0 else nc.scalar
                    eng.dma_start(out=wo_sb[:, hh, :], in_=woT_v[:, hh, :])

                pending = []

                def emit_outproj(q0o, qs, ots):
                    for dc in range(4):
                        d0 = dc * 512
                        op = psO.tile([128, 512], F32, tag="op", name="op")
                        for h in range(HC):
                            nc.tensor.matmul(
                                op[:],
                                ots[h][:, qs * 128:(qs + 1) * 128],
                                wo_sb[:, h, d0:d0 + 512],
                                start=(h == 0),
                                stop=(h == HC - 1),
                            )
                        outt = p3out.tile([128, 512], F32, tag="outt",
                                          name="outt")
                        nc.vector.tensor_copy(outt[:], op[:])
                        nc.sync.dma_start(
                            out=out_d[q0o + qs * 128:q0o + (qs + 1) * 128,
                                      d0:d0 + 512],
                            in_=outt[:],
                        )

                for qt in range(NTQ):
                    q0 = qt * TQ
                    # bias for this query tile, both key halves
                    bias_h = [None, None]
                    for half in range(2):
                        if qt == 0 and half == 0:
                            bias_h[0] = bias0  # prefetched during pass A
                            continue
                        pool = p3b0 if half == 0 else p3b1
                        bc = pool.tile([128, NHF, TQ], BF16, tag=f"b{half}")
                        nc.gpsimd.dma_start(
                            out=bc[:],
                            in_=biasT_v[:, half * NHF:(half + 1) * NHF,
                                        q0:q0 + TQ],
                        )
                        bias_h[half] = bc

                    ot_tiles = []
                    for h in range(HC):
                        if pending:
                            emit_outproj(*pending.pop(0))
                        qcol = q_sb[:, h, q0:q0 + TQ]
                        s_buf = p3s.tile([128, NTK, TQ], F32, tag="s")
                        # scores: stationary kT blocks, psum singles -> s_buf
                        for kb in range(NTK):
                            sps = psS.tile([128, TQ], F32, tag="sps")
                            nc.tensor.matmul(
                                sps[:],
                                kT_sb[:, h, kb * 128:(kb + 1) * 128],
                                qcol,
                                start=True,
                                stop=True,
                            )
                            bh = bias_h[kb // NHF]
                            nc.vector.tensor_add(
                                s_buf[:, kb, :],
                                sps[:],
                                bh[:, kb % NHF, :],
                            )
                        av = psAZ.tile([128, TQ], F32, tag="av")
                        zb = psAZ.tile([128, TQ], F32, tag="zb")
                        for half in range(2):
                            hs = slice(half * NHF, (half + 1) * NHF)
                            s_flat = s_buf[:, hs, :].rearrange(
                                "p a b -> p (a b)")
                            nc.scalar.activation(
                                s_flat, s_flat,
                                mybir.ActivationFunctionType.Tanh,
                                scale=1.0 / cap,
                            )
                            er = p3er.tile([128, NHF, TQ], BF16, tag="er")
                            nc.scalar.activation(
                                er[:].rearrange("p a b -> p (a b)"),
                                s_flat,
                                mybir.ActivationFunctionType.Exp,
                                scale=cap,
                            )
                            for kk in range(NHF):
                                kb = half * NHF + kk
                                nc.tensor.matmul(
                                    av[:],
                                    v_sb[:, kb, h * DK:(h + 1) * DK],
                                    er[:, kk, :],
                                    start=(kb == 0),
                                    stop=(kb == NTK - 1),
                                )
                            # denominator pair-tree on DVE: 8 -> 2 blocks
                            er2 = p3et.tile([128, 4, TQ], BF16, tag="er2")
                            nc.vector.tensor_add(
                                er2[:], er[:, 0:NHF:2, :], er[:, 1:NHF:2, :])
                            er4 = p3et.tile([128, 2, TQ], BF16, tag="er4")
                            nc.vector.tensor_add(
                                er4[:], er2[:, 0:4:2, :], er2[:, 1:4:2, :])
                            for j in range(2):
                                nc.tensor.matmul(
                                    zb[:],
                                    ones_full[:],
                                    er4[:, j, :],
                                    start=(half == 0 and j == 0),
                                    stop=(half == 1 and j == 1),
                                )
                        recip = p3recip.tile([128, TQ], F32, tag="recip")
                        nc.vector.reciprocal_approx_fast(
                            out=recip[:], in_=zb[:])
                        ot = p3ot.tile([128, TQ], BF16, tag="ot")
                        nc.vector.tensor_mul(ot[:], av[:], recip[:])
                        ot_tiles.append(ot)

                    # defer output projection; interleave with next tile
                    for qs in range(TQ // 128):
                        pending.append((q0, qs, ot_tiles))
                for item in pending:
                    emit_outproj(*item)

            pbias.__exit__(None, None, None)

    nc.compile()
    return nc


_PROGRAM_CACHE: dict = {}


def _get_program(cap: float):
    if cap not in _PROGRAM_CACHE:
        _PROGRAM_CACHE[cap] = _build_program(cap)
    return _PROGRAM_CACHE[cap]


def _prepare_in_maps(inp, wq, wk, wv, wo, attn_bias, softcap):
    inp = np.asarray(inp, dtype=np.float32)
    xTs = [
        np.ascontiguousarray(inp[b].T).astype(_bf16) for b in range(B)
    ]
    biasT = np.ascontiguousarray(
        np.asarray(attn_bias, dtype=np.float32).reshape(S, S).T
    ).astype(_bf16)
    wq = np.asarray(wq, dtype=np.float32)
    wk = np.asarray(wk, dtype=np.float32)
    wv = np.asarray(wv, dtype=np.float32)
    wo = np.asarray(wo, dtype=np.float32)
    scale = 1.0 / np.sqrt(np.float32(DK))

    in_maps = []
    for c in range(NCORES):
        b = c // 4
        g = c % 4
        rows = slice(g * DPC, (g + 1) * DPC)
        in_maps.append({
            "xT": xTs[b],
            "ones": np.ones((128, 128), dtype=_bf16),
            "wqT": np.ascontiguousarray((wq[rows] * scale).T).astype(_bf16),
            "wkT": np.ascontiguousarray(wk[rows].T).astype(_bf16),
            "wvT": np.ascontiguousarray(wv[rows].T).astype(_bf16),
            "woT": np.ascontiguousarray(wo[:, rows].T).astype(_bf16),
            "biasT": biasT,
        })
    return in_maps


def run(inputs: dict, trace: bool = False):
    """Run the SPMD kernel. Returns (full_output, BassKernelResults)."""
    cap = float(inputs["softcap"])
    nc = _get_program(cap)
    in_maps = _prepare_in_maps(
        inputs["inp"], inputs["wq"], inputs["wk"], inputs["wv"],
        inputs["wo"], inputs["attn_bias"], inputs["softcap"],
    )
    res = run_bass_kernel_spmd(
        nc, in_maps, list(range(NCORES)), trace=trace,
    )
    out = np.zeros((B, S, D), dtype=np.float64)
    for c in range(NCORES):
        out[c // 4] += res.results[c]["out_partial"]
    return out.astype(np.float32), res


def kernel(**inputs) -> np.ndarray:
    out, _ = run(inputs, trace=False)
    return out


if __name__ == "__main__":
    rng = np.random.default_rng(0)
    sc = 1.0 / np.sqrt(D)
    inputs = {
        "inp": rng.standard_normal((B, S, D)).astype(np.float32),
        "wq": (rng.standard_normal((D, D)) * sc).astype(np.float32),
        "wk": (rng.standard_normal((D, D)) * sc).astype(np.float32),
        "wv": (rng.standard_normal((D, D)) * sc).astype(np.float32),
        "wo": (rng.standard_normal((D, D)) * sc).astype(np.float32),
        "attn_bias": rng.standard_normal((1, 1, S, S)).astype(np.float32),
        "softcap": 30,
    }
    out = kernel(**inputs)
    print("out", out.shape, out.dtype, float(np.abs(out).max()))
